# revision 36
# baseline (speedup 1.0000x reference)
"""CLAHE effect kernel for Trainium2 (8 NeuronCores, Bass/Tile).

Sharding: core c gets image rows [512c, 512c+512) = tile-row c of the 8x8
CLAHE grid; all 8 tiles of that row are fully local (histogram, CDF, remap),
so no collectives are needed. kernel() takes full inputs, shards host-side,
runs the SPMD bass kernel, and reassembles the full output.

Per-core pipeline (strip [3, 512, 4096] f32, 8 tiles of 512x512):
  Pass 1 (per tile): lum = mean(rgb); u1 = min(lum*256, 255.5); r1 = u1 mod 16
    staircase planes SA_h = [u1 >= 16h], SB_l = [r1 >= l]  (bf16, 16+16)
    G[h,l] = sum_px SA_h*SB_l via PE matmuls, PSUM-accumulated
           = #{px : hi >= h and lo >= l};  per-tile min/max of lum.
  Mid (all tiles batched on (tile,l) partition groups):
    hist = 2D finite difference of G; excess clip; cdf via PE triangular
    matmuls + small scans; alpha-fold; broadcast 256-entry tables + per-tile
    scalars to all partitions.
  Pass 2 (per tile): q = (lum-tmin)/d computed to ~2^-45 via rcp-mult +
    Veltkamp/Dekker exact-residual Newton (bit-matches true fp32 divide on
    the bin decision); L = alpha*cdf[idx] via exact stair sweep with masks
    on ScalarE:  s_b = Sign(q - psi_b^-)  (ACT, parallel engine)
                 acc = max(acc, s_b * T[b])  (one fused DVE stt per bin)
    psi_b = precomputed fp32 thresholds folding the *255 rounding.
    enh = L + (1-alpha)*lum;  out_c = clip(enh * (1/lum) * img_c, 0, 1).
"""

import numpy as np

G = 8
BINS = 256
H = W = 4096
HS = WS = H // G          # 512
P = 128
RB = HS // P              # 4 row-blocks
FREE = RB * WS            # 2048
CH = 512                  # staircase column chunk
NCH = FREE // CH

_COMPILED = None


def _psi_table():
    """psi[b] = smallest fp32 q with RN(255*q) >= b."""
    psi = np.zeros(256, np.float32)
    for b in range(256):
        q = np.float32(b / 255.0)
        while np.float32(255.0) * q >= b:
            q = np.nextafter(q, np.float32(-1), dtype=np.float32)
        while np.float32(255.0) * q < b:
            q = np.nextafter(q, np.float32(2), dtype=np.float32)
        psi[b] = q
    return psi


def _psi_minus_table():
    """nextafter-down(psi): [q >= psi[b]] <=> [q > psim[b]] <=> Sign(q-psim)>0."""
    psi = _psi_table()
    return np.nextafter(psi, np.float32(-1), dtype=np.float32)


def _build():
    import contextlib
    import concourse.bass as bass
    import concourse.bacc as bacc
    import concourse.tile as tile
    import concourse.mybir as mybir
    from concourse.alu_op_type import AluOpType as Op

    PSI = _psi_table()
    PSIM = _psi_minus_table()
    dt = mybir.dt
    f32 = dt.float32
    bf16 = dt.bfloat16
    nc = bacc.Bacc("TRN2", target_bir_lowering=False, debug=False,
                   num_devices=G)

    img = nc.dram_tensor("img", [3, HS, W], f32, kind="ExternalInput").ap()
    alf = nc.dram_tensor("alf", [1, G], f32, kind="ExternalInput").ap()
    out = nc.dram_tensor("out", [3, HS, W], f32, kind="ExternalOutput").ap()

    scr_tab = nc.dram_tensor("scr_tab", [G, 16, 16], f32)  # (t, h, l)
    scr_pt = nc.dram_tensor("scr_pt", [8, 8], f32)
    scr_mm = nc.dram_tensor("scr_mm", [P, 2 * G], f32)

    img_rb = img.rearrange("c (rb p) w -> c rb p w", p=P)
    out_rb = out.rearrange("c (rb p) w -> c rb p w", p=P)

    # constants
    eye16 = np.eye(16, dtype=np.float32)
    ETILE = nc.inline_tensor(np.tile(eye16, (8, 1)), "ETILE")          # [128,16]
    LTRI = nc.inline_tensor(
        np.kron(np.eye(8, dtype=np.float32),
                np.triu(np.ones((16, 16), np.float32))), "LTRI")       # [128,128]
    r15 = np.zeros((16, 16), np.float32)
    r15[15, :] = 1.0
    PICK15 = nc.inline_tensor(
        np.kron(np.eye(8, dtype=np.float32), r15), "PICK15")           # [128,128]
    TS8 = nc.inline_tensor(
        np.kron(np.eye(8, dtype=np.float32), np.ones((16, 1), np.float32)),
        "TS8")                                                          # [128,8]
    BC16 = nc.inline_tensor(
        np.kron(np.eye(8, dtype=np.float32), np.ones((1, 16), np.float32)),
        "BC16")                                                         # [8,128]
    ID16 = nc.inline_tensor(eye16, "ID16")
    NPSIM = nc.inline_tensor(
        np.tile(-_psi_minus_table().reshape(1, -1), (P, 1)), "NPSIM")  # [128,256]

    with tile.TileContext(nc) as tc, contextlib.ExitStack() as ctx:
        cpool = ctx.enter_context(tc.tile_pool(name="consts", bufs=1))
        e_t = cpool.tile([P, 16], f32)
        nc.sync.dma_start(e_t[:], ETILE.ap())
        ltri_t = cpool.tile([P, P], f32)
        nc.sync.dma_start(ltri_t[:], LTRI.ap())
        p15_t = cpool.tile([P, P], f32)
        nc.sync.dma_start(p15_t[:], PICK15.ap())
        ts8_t = cpool.tile([P, 8], f32)
        nc.sync.dma_start(ts8_t[:], TS8.ap())
        bc16_t = cpool.tile([8, P], f32)
        nc.sync.dma_start(bc16_t[:], BC16.ap())
        id16_t = cpool.tile([16, 16], f32)
        nc.sync.dma_start(id16_t[:], ID16.ap())
        npsim_t = cpool.tile([P, BINS], f32)
        nc.sync.dma_start(npsim_t[:], NPSIM.ap())

        small = ctx.enter_context(tc.tile_pool(name="small", bufs=1))
        mins_all = small.tile([P, G], f32, tag="mins")
        maxs_all = small.tile([P, G], f32, tag="maxs")

        pspool = ctx.enter_context(tc.tile_pool(name="ps", bufs=1, space="PSUM"))
        gps_all = pspool.tile([P, G * P], f32, tag="gpsall", name="gps_all")
        gpsums = [gps_all[:, t * P:(t + 1) * P] for t in range(G)]

        # ---------------- PASS 1 ----------------
        with tc.tile_pool(name="p1in", bufs=2) as p1in, \
             tc.tile_pool(name="p1work", bufs=1) as p1w, \
             tc.tile_pool(name="stairs", bufs=2) as stp:
            for t in range(G):
                chs = []
                for c in range(3):
                    cht = p1in.tile([P, FREE], f32, tag=f"in{c}")
                    nc.sync.dma_start(
                        cht[:].rearrange("p (rb w) -> p rb w", rb=RB),
                        img_rb[c, :, :, t * WS:(t + 1) * WS].rearrange(
                            "rb p w -> p rb w"))
                    chs.append(cht)
                lum = p1w.tile([P, FREE], f32, tag="lum")
                nc.vector.tensor_tensor(lum[:], chs[0][:], chs[1][:], Op.add)
                nc.vector.tensor_tensor(lum[:], lum[:], chs[2][:], Op.add)
                nc.vector.tensor_scalar(lum[:], lum[:], 1.0 / 3.0, None,
                                        Op.mult)
                nc.vector.tensor_reduce(mins_all[:, t:t + 1], lum[:],
                                        mybir.AxisListType.X, Op.min)
                nc.vector.tensor_reduce(maxs_all[:, t:t + 1], lum[:],
                                        mybir.AxisListType.X, Op.max)
                u1 = p1w.tile([P, FREE], f32, tag="u1")
                nc.vector.tensor_scalar(u1[:], lum[:], 256.0, 255.5, Op.mult,
                                        Op.min)
                # exact floor(u1): cast rounds-to-nearest on HW, so correct
                # with fl = cast_back - (cast_back > u1)
                i32 = p1w.tile([P, FREE], dt.int32, tag="i32")
                nc.vector.tensor_copy(i32[:], u1[:])
                fb = p1w.tile([P, FREE], f32, tag="fb")
                nc.vector.tensor_copy(fb[:], i32[:])
                co = p1w.tile([P, FREE], f32, tag="co")
                nc.vector.tensor_tensor(co[:], fb[:], u1[:], Op.is_gt)
                fl = p1w.tile([P, FREE], f32, tag="fl")
                nc.vector.tensor_tensor(fl[:], fb[:], co[:], Op.subtract)
                # w = fl/16 (exact); hi = floor(w) via same corrected cast
                w_t = p1w.tile([P, FREE], f32, tag="w_t")
                nc.vector.tensor_scalar(w_t[:], fl[:], 1.0 / 16.0, None,
                                        Op.mult)
                nc.vector.tensor_copy(i32[:], w_t[:])
                nc.vector.tensor_copy(fb[:], i32[:])
                nc.vector.tensor_tensor(co[:], fb[:], w_t[:], Op.is_gt)
                nc.vector.tensor_tensor(fb[:], fb[:], co[:], Op.subtract)
                # r1 = fl - 16*hi
                r1 = p1w.tile([P, FREE], f32, tag="r1")
                nc.vector.scalar_tensor_tensor(r1[:], fb[:], -16.0, fl[:],
                                               Op.mult, Op.add)

                gp = gpsums[t]
                for ci in range(NCH):
                    sa = stp.tile([P, CH, 16], bf16, tag="sa")
                    sb = stp.tile([P, CH, 16], bf16, tag="sb")
                    for h in range(16):
                        nc.vector.tensor_scalar(
                            sa[:, :, h], u1[:, ci * CH:(ci + 1) * CH],
                            float(16 * h), None, Op.is_ge)
                        nc.vector.tensor_scalar(
                            sb[:, :, h], r1[:, ci * CH:(ci + 1) * CH],
                            float(h), None, Op.is_ge)
                    ng = CH // 8
                    for g_i in range(ng):
                        lhsT = sa[:, g_i * 8:(g_i + 1) * 8, :].rearrange(
                            "p w h -> p (w h)")
                        rhs = sb[:, g_i * 8:(g_i + 1) * 8, :].rearrange(
                            "p w h -> p (w h)")
                        nc.tensor.matmul(
                            gp, lhsT, rhs,
                            start=(ci == 0 and g_i == 0),
                            stop=(ci == NCH - 1 and g_i == ng - 1))

        # ---------------- MID ----------------
        stacked = small.tile([P, P], f32, tag="stacked")
        gsb = small.tile([P, G * P], f32, tag="gsb")
        for t in range(G):
            nc.scalar.copy(gsb[:, t * P:(t + 1) * P], gpsums[t])
        for t in range(G):
            for c in range(8):
                nc.sync.dma_start(
                    stacked[16 * c:16 * (c + 1), 16 * t:16 * (t + 1)],
                    gsb[16 * c:16 * (c + 1),
                        t * P + 16 * c:t * P + 16 * (c + 1)])
        gstack_ps = pspool.tile([16, P], f32, tag="midps")
        nc.tensor.matmul(gstack_ps[:], e_t[:], stacked[:], start=True,
                         stop=True)
        gpad = small.tile([16, 8 * 17], f32, tag="gpad")
        nc.vector.memset(gpad[:], 0.0)
        nc.scalar.copy(
            gpad[:].rearrange("p (t l) -> p t l", t=8)[:, :, 0:16],
            gstack_ps[:].rearrange("p (t l) -> p t l", t=8))
        dmat = small.tile([16, P], f32, tag="dmat")
        gv = gpad[:].rearrange("p (t l) -> p t l", t=8)
        nc.vector.tensor_tensor(
            dmat[:].rearrange("p (t l) -> p t l", t=8),
            gv[:, :, 0:16], gv[:, :, 1:17], Op.subtract)
        dT_ps = pspool.tile([P, 16], f32, tag="midps")
        nc.tensor.transpose(dT_ps[:], dmat[:], id16_t[:])
        dTpad = small.tile([P, 17], f32, tag="dTpad")
        nc.vector.memset(dTpad[:, 16:17], 0.0)
        nc.scalar.copy(dTpad[:, 0:16], dT_ps[:])
        histT = small.tile([P, 16], f32, tag="histT")   # [(t,l), h]
        nc.vector.tensor_tensor(histT[:], dTpad[:, 0:16], dTpad[:, 1:17],
                                Op.subtract)

        relu16 = small.tile([P, 16], f32, tag="relu16")
        nc.vector.tensor_scalar(relu16[:], histT[:], 4096.0, 0.0, Op.subtract,
                                Op.max)
        rowsum = small.tile([P, 1], f32, tag="rowsum")
        nc.vector.tensor_reduce(rowsum[:], relu16[:], mybir.AxisListType.X,
                                Op.add)
        ex8_ps = pspool.tile([8, 1], f32, tag="midps")
        nc.tensor.matmul(ex8_ps[:], ts8_t[:], rowsum[:], start=True, stop=True)
        ex8 = small.tile([8, 1], f32, tag="ex8s")
        nc.scalar.copy(ex8[:], ex8_ps[:])
        exb_ps = pspool.tile([P, 1], f32, tag="midps")
        nc.tensor.matmul(exb_ps[:], bc16_t[:], ex8[:], start=True, stop=True)
        exs = small.tile([P, 1], f32, tag="exs")
        nc.vector.tensor_scalar(exs[:], exb_ps[:], 1.0 / 256.0, None, Op.mult)
        histc = small.tile([P, 16], f32, tag="histc")
        nc.vector.tensor_scalar(histc[:], histT[:], 4096.0, None, Op.min)
        nc.vector.tensor_scalar(histc[:], histc[:], exs[:], None, Op.add)

        w_ps = pspool.tile([P, 16], f32, tag="midps")
        nc.tensor.matmul(w_ps[:], ltri_t[:], histc[:], start=True, stop=True)
        ws = small.tile([P, 16], f32, tag="ws")
        nc.scalar.copy(ws[:], w_ps[:])
        sb_ps = pspool.tile([P, 16], f32, tag="midps")
        nc.tensor.matmul(sb_ps[:], p15_t[:], ws[:], start=True, stop=True)
        sbs = small.tile([P, 16], f32, tag="sbs")
        nc.scalar.copy(sbs[:], sb_ps[:])
        # exclusive prefix over h (free dim, 16): shift then Hillis-Steele
        pref = small.tile([P, 16], f32, tag="pref")
        nc.vector.memset(pref[:], 0.0)
        nc.scalar.copy(pref[:, 1:16], sbs[:, 0:15])
        sh = small.tile([P, 16], f32, tag="sh")
        for s in (1, 2, 4, 8):
            nc.vector.memset(sh[:], 0.0)
            nc.scalar.copy(sh[:, s:16], pref[:, 0:16 - s])
            nc.vector.tensor_tensor(pref[:], pref[:], sh[:], Op.add)
        cdfT = small.tile([P, 16], f32, tag="cdfT")
        nc.vector.tensor_tensor(cdfT[:], ws[:], pref[:], Op.add)
        nc.vector.tensor_scalar(cdfT[:], cdfT[:], 1.0 / 262144.0, None,
                                Op.mult)

        # per-tile scalars on 8 partitions
        nc.sync.dma_start(scr_mm.ap()[:, 0:G], mins_all[:])
        nc.sync.dma_start(scr_mm.ap()[:, G:2 * G], maxs_all[:])
        minT = small.tile([G, P], f32, tag="minT")
        nc.sync.dma_start(minT[:], scr_mm.ap()[:, 0:G].rearrange("p t -> t p"))
        maxT = small.tile([G, P], f32, tag="maxT")
        nc.sync.dma_start(maxT[:],
                          scr_mm.ap()[:, G:2 * G].rearrange("p t -> t p"))
        tmin8 = small.tile([G, 1], f32, tag="tmin8")
        nc.vector.tensor_reduce(tmin8[:], minT[:], mybir.AxisListType.X,
                                Op.min)
        tmax8 = small.tile([G, 1], f32, tag="tmax8")
        nc.vector.tensor_reduce(tmax8[:], maxT[:], mybir.AxisListType.X,
                                Op.max)
        d8 = small.tile([G, 1], f32, tag="d8")
        nc.vector.tensor_tensor(d8[:], tmax8[:], tmin8[:], Op.subtract)
        v8 = small.tile([G, 1], f32, tag="v8")
        nc.vector.tensor_scalar(v8[:], d8[:], 0.0, None, Op.is_gt)
        omv8 = small.tile([G, 1], f32, tag="omv8")
        nc.vector.tensor_scalar(omv8[:], v8[:], -1.0, 1.0, Op.mult, Op.add)
        sd8 = small.tile([G, 1], f32, tag="sd8")
        nc.vector.tensor_tensor(sd8[:], d8[:], v8[:], Op.mult)
        nc.vector.tensor_tensor(sd8[:], sd8[:], omv8[:], Op.add)
        rcp8 = small.tile([G, 1], f32, tag="rcp8")
        nc.vector.reciprocal(rcp8[:], sd8[:])
        # Veltkamp split of d: dh + dl == d exactly
        dt1 = small.tile([G, 1], f32, tag="dt1")
        nc.vector.tensor_scalar(dt1[:], sd8[:], 4097.0, None, Op.mult)
        dt2 = small.tile([G, 1], f32, tag="dt2")
        nc.vector.tensor_tensor(dt2[:], dt1[:], sd8[:], Op.subtract)
        dh8 = small.tile([G, 1], f32, tag="dh8")
        nc.vector.tensor_tensor(dh8[:], dt1[:], dt2[:], Op.subtract)
        dl8 = small.tile([G, 1], f32, tag="dl8")
        nc.vector.tensor_tensor(dl8[:], sd8[:], dh8[:], Op.subtract)
        alf8 = small.tile([G, 1], f32, tag="alf8")
        nc.sync.dma_start(alf8[:], alf.rearrange("a g -> g a"))
        a8 = small.tile([G, 1], f32, tag="a8")
        nc.vector.tensor_scalar(a8[:], alf8[:], 0.5, 0.5, Op.mult, Op.add)
        nc.vector.tensor_tensor(a8[:], a8[:], v8[:], Op.mult)
        oma8 = small.tile([G, 1], f32, tag="oma8")
        nc.vector.tensor_scalar(oma8[:], a8[:], -1.0, 1.0, Op.mult, Op.add)

        # alpha-fold into tables: ab[(t,l)] = a8[t]
        ab_ps = pspool.tile([P, 1], f32, tag="midps")
        nc.tensor.matmul(ab_ps[:], bc16_t[:], a8[:], start=True, stop=True)
        ab = small.tile([P, 1], f32, tag="ab")
        nc.scalar.copy(ab[:], ab_ps[:])
        nc.vector.tensor_scalar(cdfT[:], cdfT[:], ab[:], None, Op.mult)

        # tables -> dram in (t, h, l) flat order: one DMA per h
        for hh in range(16):
            nc.sync.dma_start(scr_tab.ap()[:, hh, :], cdfT[:, hh:hh + 1])
        pt8 = small.tile([8, 8], f32, tag="pt8")
        nc.vector.memset(pt8[:], 0.0)
        nc.scalar.copy(pt8[:, 0:1], tmin8[:])
        nc.scalar.copy(pt8[:, 1:2], rcp8[:])
        nc.scalar.copy(pt8[:, 2:3], oma8[:])
        nc.scalar.copy(pt8[:, 3:4], dh8[:])
        nc.scalar.copy(pt8[:, 4:5], dl8[:])
        nc.sync.dma_start(scr_pt.ap(), pt8[:])

        sweeppool = ctx.enter_context(tc.tile_pool(name="sweep", bufs=1))
        tabpool = ctx.enter_context(tc.tile_pool(name="tabs", bufs=1))
        tables = tabpool.tile([P, G * BINS], f32, tag="tables")
        nc.sync.dma_start(
            tables[:],
            scr_tab.ap().rearrange("t h l -> (t h l)").unsqueeze(0)
            .partition_broadcast(P))
        ptb = tabpool.tile([P, 64], f32, tag="ptb")
        nc.sync.dma_start(
            ptb[:], scr_pt.ap().rearrange("t s -> (t s)").unsqueeze(0).partition_broadcast(P))

        # ---------------- PASS 2 ----------------
        with tc.tile_pool(name="p2in", bufs=2) as p2in, \
             tc.tile_pool(name="p2out", bufs=1) as p2out, \
             tc.tile_pool(name="signs", bufs=3) as signpool:
            for t in range(G):
                tbl = tables[:, BINS * t:BINS * (t + 1)]
                tmin_c = ptb[:, 8 * t + 0:8 * t + 1]
                rcp_c = ptb[:, 8 * t + 1:8 * t + 2]
                oma_c = ptb[:, 8 * t + 2:8 * t + 3]
                dh_c = ptb[:, 8 * t + 3:8 * t + 4]
                dl_c = ptb[:, 8 * t + 4:8 * t + 5]

                chs = []
                for c in range(3):
                    cht = p2in.tile([P, FREE], f32, tag=f"in{c}")
                    nc.sync.dma_start(
                        cht[:].rearrange("p (rb w) -> p rb w", rb=RB),
                        img_rb[c, :, :, t * WS:(t + 1) * WS].rearrange(
                            "rb p w -> p rb w"))
                    chs.append(cht)
                lum = sweeppool.tile([P, FREE], f32, tag="lum2")
                nc.vector.tensor_tensor(lum[:], chs[0][:], chs[1][:], Op.add)
                nc.vector.tensor_tensor(lum[:], lum[:], chs[2][:], Op.add)
                nc.vector.tensor_scalar(lum[:], lum[:], 1.0 / 3.0, None,
                                        Op.mult)

                x_t = sweeppool.tile([P, FREE], f32, tag="xt")
                nc.vector.tensor_scalar(x_t[:], lum[:], tmin_c, None,
                                        Op.subtract)
                q0 = sweeppool.tile([P, FREE], f32, tag="q0")
                nc.vector.tensor_scalar(q0[:], x_t[:], rcp_c, None, Op.mult)
                # Veltkamp split of q0; exact residual r = x - q0*d; q1 newton
                s1 = sweeppool.tile([P, FREE], f32, tag="s1")
                nc.vector.tensor_scalar(s1[:], q0[:], 4097.0, None, Op.mult)
                s2 = sweeppool.tile([P, FREE], f32, tag="mb")
                nc.vector.tensor_tensor(s2[:], s1[:], q0[:], Op.subtract)
                q0h = sweeppool.tile([P, FREE], f32, tag="q0h")
                nc.vector.tensor_tensor(q0h[:], s1[:], s2[:], Op.subtract)
                q0l = sweeppool.tile([P, FREE], f32, tag="q0l")
                nc.vector.tensor_tensor(q0l[:], q0[:], q0h[:], Op.subtract)
                r_t = sweeppool.tile([P, FREE], f32, tag="rt")
                nc.vector.tensor_scalar(s1[:], q0h[:], dh_c, None, Op.mult)
                nc.vector.tensor_tensor(r_t[:], x_t[:], s1[:], Op.subtract)
                nc.vector.tensor_scalar(s1[:], q0h[:], dl_c, None, Op.mult)
                nc.vector.tensor_tensor(r_t[:], r_t[:], s1[:], Op.subtract)
                nc.vector.tensor_scalar(s1[:], q0l[:], dh_c, None, Op.mult)
                nc.vector.tensor_tensor(r_t[:], r_t[:], s1[:], Op.subtract)
                nc.vector.tensor_scalar(s1[:], q0l[:], dl_c, None, Op.mult)
                nc.vector.tensor_tensor(r_t[:], r_t[:], s1[:], Op.subtract)
                nc.vector.tensor_scalar(s1[:], r_t[:], rcp_c, None, Op.mult)
                q1 = sweeppool.tile([P, FREE], f32, tag="q1")
                nc.vector.tensor_tensor(q1[:], q0[:], s1[:], Op.add)

                acc = sweeppool.tile([P, FREE], f32, tag="acc")
                nc.vector.tensor_scalar(acc[:], q1[:], 0.0, tbl[:, 0:1],
                                        Op.is_ge, Op.mult)
                acc2 = sweeppool.tile([P, FREE], f32, tag="acc2")
                nc.vector.tensor_scalar(acc2[:], q1[:], float(PSI[128]),
                                        tbl[:, 128:129], Op.is_ge, Op.mult)
                for b in range(1, 128):
                    for bb, at in ((b, acc), (b + 128, acc2)):
                        if bb == 128:
                            continue
                        sgn = signpool.tile([P, FREE], f32, tag="sgn",
                                            name=f"sgn{t}_{bb}")
                        nc.scalar.sign(sgn[:], q1[:], npsim_t[:, bb:bb + 1])
                        nc.vector.scalar_tensor_tensor(
                            at[:], sgn[:], tbl[:, bb:bb + 1], at[:],
                            Op.mult, Op.max)
                sgn = signpool.tile([P, FREE], f32, tag="sgn",
                                    name=f"sgn{t}_255")
                nc.scalar.sign(sgn[:], q1[:], npsim_t[:, 255:256])
                nc.vector.scalar_tensor_tensor(
                    acc2[:], sgn[:], tbl[:, 255:256], acc2[:],
                    Op.mult, Op.max)
                nc.vector.tensor_tensor(acc[:], acc[:], acc2[:], Op.max)

                enh = sweeppool.tile([P, FREE], f32, tag="xt")
                nc.vector.scalar_tensor_tensor(enh[:], lum[:], oma_c, acc[:],
                                               Op.mult, Op.add)
                rcp_l = sweeppool.tile([P, FREE], f32, tag="q0")
                nc.vector.reciprocal(rcp_l[:], lum[:])
                q_t = sweeppool.tile([P, FREE], f32, tag="q0h")
                nc.vector.tensor_tensor(q_t[:], enh[:], rcp_l[:], Op.mult)

                for c in range(3):
                    o_t = p2out.tile([P, FREE], f32, tag=f"o{c}")
                    nc.vector.tensor_tensor(o_t[:], q_t[:], chs[c][:], Op.mult)
                    nc.vector.tensor_scalar(o_t[:], o_t[:], 0.0, 1.0, Op.max,
                                            Op.min)
                    nc.sync.dma_start(
                        out_rb[c, :, :, t * WS:(t + 1) * WS].rearrange(
                            "rb p w -> p rb w"),
                        o_t[:].rearrange("p (rb w) -> p rb w", rb=RB))

    nc.compile()
    return nc


LAST_EXEC_NS = None


def kernel(img: np.ndarray, alphas: np.ndarray, trace: bool = False) -> np.ndarray:
    global _COMPILED, LAST_EXEC_NS
    from concourse.bass_utils import run_bass_kernel_spmd
    if _COMPILED is None:
        _COMPILED = _build()
    nc = _COMPILED
    img = np.asarray(img, dtype=np.float32)
    alphas = np.asarray(alphas, dtype=np.float32)
    in_maps = []
    for c in range(G):
        in_maps.append({
            "img": np.ascontiguousarray(img[:, c * HS:(c + 1) * HS, :]),
            "alf": np.ascontiguousarray(
                alphas[c * G:(c + 1) * G].reshape(1, G)),
        })
    res = run_bass_kernel_spmd(nc, in_maps, list(range(G)), trace=trace)
    if res.exec_time_ns is not None:
        LAST_EXEC_NS = res.exec_time_ns
    out = np.empty((3, H, W), np.float32)
    for c in range(G):
        out[:, c * HS:(c + 1) * HS, :] = res.results[c]["out"]
    return out


if __name__ == "__main__":
    rng = np.random.default_rng(0)
    img = rng.random((3, H, W), dtype=np.float32)
    alphas = rng.random(64, dtype=np.float32)
    o = kernel(img, alphas)
    print("ran", o.shape, o.dtype)
